# revision 1
# baseline (speedup 1.0000x reference)
"""TRN2 Bass kernel for nn_LocalAttention (B=4, T=2048, C=1024, window=16).

Sharding: 8 cores = (batch b, row-half h). Each core computes K^T/V for its
whole batch (duplicated across the 2 cores of a batch) and attention +
projections for its own 1024 rows (two 512-row chunks; h=0 gets global
chunks {0,3}, h=1 gets {1,2}; slot 0 = denser chunk).

All matmuls run in fp32r (TF32-like, ~1.5e-4 rel err, 4x fp32 speed). Raw
fp32 bytes are declared as fp32r at the DRAM boundary - the PE rounds
internally (validated: identical error to explicit cast-DMA).

Orientation trick: host passes X^T and W^T so every matmul is natural:
  K^T = (Wk^T)^T @ X^T        [C, T]     (DRAM scratch)
  V   = (X^T)^T @ Wv^T        [T, C]     (DRAM scratch)
  Q^T = (Wq^T)^T @ X_own^T    [C, 1024]  (SBUF resident)
  S^T = (K^T_blk)^T @ Q^T_chunk  -> [keys, rows]; softmax-over-keys is a
        partition reduction done by a ones-vector matmul, and E^T feeds
  Y^T = V_blk^T @ E^T            [C, rows]
  Z^T = (Wo^T)^T @ Y^T           [C, rows]

Sparsity: mask keeps j >= i - 16 (reverse-causal), so each 512-row chunk's
kept key-block set is a SUFFIX {b..15}; processing key blocks in descending
order (position p -> block 15-p) makes every kept set a static PREFIX.
Chunk slot 0 runs 16 positions, slot 1 runs 9 - uniform across cores, the
data-driven is_ge mask zeroes over-included blocks. Mask applied
multiplicatively post-exp (scores are O(6), no overflow without max-sub).
"""
import numpy as np

import concourse.bass as bass
import concourse.mybir as mybir
import concourse.tile as tile
from concourse import bacc
from concourse import bass_utils

N_CORES = 8
B, T, C = 4, 2048, 1024
WINDOW = 16
TOWN = T // 2          # own rows per core
CHUNK = 512            # rows per processing chunk
NCHUNK = TOWN // CHUNK  # 2
CI = C // 128          # 8 contraction blocks
CO = C // 128          # 8 output blocks
KB = T // 128          # 16 key blocks
TCH = T // CHUNK       # 4 t-chunks in phase A
SLOT_KBS = (16, 9)     # key-block positions per chunk slot (descending order)
F32 = mybir.dt.float32
F32R = mybir.dt.float32r

_NC_CACHE = {}


def build():
    if "nc" in _NC_CACHE:
        return _NC_CACHE["nc"]
    nc = bacc.Bacc("TRN2", target_bir_lowering=False, debug=False,
                   num_devices=N_CORES)
    xt = nc.dram_tensor("xt", [C, T], F32R, kind="ExternalInput").ap()
    xtq = nc.dram_tensor("xtq", [C, TOWN], F32R, kind="ExternalInput").ap()
    wqt = nc.dram_tensor("wqt", [C, C], F32R, kind="ExternalInput").ap()
    wkt = nc.dram_tensor("wkt", [C, C], F32R, kind="ExternalInput").ap()
    wvt = nc.dram_tensor("wvt", [C, C], F32R, kind="ExternalInput").ap()
    wot = nc.dram_tensor("wot", [C, C], F32R, kind="ExternalInput").ap()
    keyidx16 = nc.dram_tensor("keyidx16", [128, KB], F32, kind="ExternalInput").ap()
    rowidxb = nc.dram_tensor("rowidxb", [128, TOWN], F32, kind="ExternalInput").ap()
    zt = nc.dram_tensor("zt", [C, TOWN], F32, kind="ExternalOutput").ap()

    xt3 = xt.rearrange("(ko ki) t -> ki ko t", ki=128)
    xtq3 = xtq.rearrange("(ko ki) t -> ki ko t", ki=128)
    w3 = {w.tensor.name: w.rearrange("(ko ki) c -> ki ko c", ki=128)
          for w in (wqt, wkt, wvt, wot)}

    inv_sqrt_c = float(1.0 / np.sqrt(C))

    with tile.TileContext(nc) as tc:
        with tc.tile_pool(name="res", bufs=1) as res, \
             tc.tile_pool(name="dram", bufs=1, space="DRAM") as dram:
            kt_d = dram.tile([128, CI, T], F32R)      # K^T  [ki, ko, t]
            v_d = dram.tile([128, KB, C], F32R)       # V    [ki, ko, c]
            qt_sb = res.tile([128, CI, TOWN], F32R, tag="qt")  # Q^T resident
            wo_sb = res.tile([128, CI, C], F32R, tag="wo")
            ki16_sb = res.tile([128, KB], F32, tag="ki16")
            nc.gpsimd.dma_start(ki16_sb[:], keyidx16[:])
            ones_row_f32 = res.tile([1, 128], F32, tag="onesrf")
            nc.vector.memset(ones_row_f32[:], 1.0)
            ones_1x128 = res.tile([1, 128], F32R, tag="o1")
            nc.vector.tensor_copy(ones_1x128[:], ones_row_f32[:])
            ones_col_f32 = res.tile([128, 1], F32, tag="onescf")
            nc.vector.memset(ones_col_f32[:], 1.0)
            ones_128x1 = res.tile([128, 1], F32R, tag="o2")
            nc.vector.tensor_copy(ones_128x1[:], ones_col_f32[:])

            # ================= Phase A: projections =========================
            with tc.tile_pool(name="wts", bufs=1) as wts, \
                 tc.tile_pool(name="xa", bufs=2) as xa, \
                 tc.tile_pool(name="stg", bufs=3) as stg, \
                 tc.tile_pool(name="ps_k", bufs=3, space="PSUM") as ps_k, \
                 tc.tile_pool(name="ps_v", bufs=2, space="PSUM") as ps_v, \
                 tc.tile_pool(name="ps_q", bufs=2, space="PSUM") as ps_q:
                wk_sb = wts.tile([128, CI, C], F32R, tag="wk")
                wv_sb = wts.tile([128, CI, C], F32R, tag="wv")
                wq_sb = wts.tile([128, CI, C], F32R, tag="wq")
                # first xt chunk before anything else on the sync queue
                xt_sbs = []
                xt_sb0 = xa.tile([128, CI, CHUNK], F32R, tag="xa")
                nc.sync.dma_start(xt_sb0[:], xt3[:, :, (TCH - 1) * CHUNK:TCH * CHUNK])
                for co in range(CO):  # per-column loads: co=0 unblocks MMs
                    nc.sync.dma_start(wk_sb[:, :, co * 128:(co + 1) * 128],
                                      w3["wkt"][:, :, co * 128:(co + 1) * 128])
                for ci in range(CI):
                    nc.scalar.dma_start(wv_sb[:, ci, :], w3["wvt"][:, ci, :])
                for ci in range(CI):
                    nc.scalar.dma_start(wq_sb[:, ci, :], w3["wqt"][:, ci, :])

                for tch in reversed(range(TCH)):
                    if tch == TCH - 1:
                        xt_sb = xt_sb0
                    else:
                        xt_sb = xa.tile([128, CI, CHUNK], F32R, tag="xa")
                        nc.sync.dma_start(
                            xt_sb[:], xt3[:, :, tch * CHUNK:(tch + 1) * CHUNK])
                    # K^T [cout, t]
                    for co in range(CO):
                        kps = ps_k.tile([128, CHUNK], F32, tag="kps")
                        for ci in range(CI):
                            nc.tensor.matmul(
                                kps[:], wk_sb[:, ci, co * 128:(co + 1) * 128],
                                xt_sb[:, ci, :], start=(ci == 0), stop=(ci == CI - 1))
                        kstage = stg.tile([128, CHUNK], F32R, tag="kstage")
                        nc.vector.tensor_copy(kstage[:], kps[:])
                        nc.sync.dma_start(
                            kt_d[:, co, tch * CHUNK:(tch + 1) * CHUNK], kstage[:])
                    # V [t, cout]
                    for tb in range(CHUNK // 128):
                        for half in range(2):
                            vps = ps_v.tile([128, 512], F32, tag="vps")
                            for ci in range(CI):
                                nc.tensor.matmul(
                                    vps[:], xt_sb[:, ci, tb * 128:(tb + 1) * 128],
                                    wv_sb[:, ci, half * 512:(half + 1) * 512],
                                    start=(ci == 0), stop=(ci == CI - 1))
                            vstage = stg.tile([128, 512], F32R, tag="vstage")
                            nc.vector.tensor_copy(vstage[:], vps[:])
                            nc.scalar.dma_start(
                                v_d[:, tch * (CHUNK // 128) + tb,
                                    half * 512:(half + 1) * 512], vstage[:])

                for qch in range(TOWN // CHUNK):
                    xq_sb = xa.tile([128, CI, CHUNK], F32R, tag="xa")
                    nc.sync.dma_start(
                        xq_sb[:], xtq3[:, :, qch * CHUNK:(qch + 1) * CHUNK])
                    for co in range(CO):
                        qps = ps_q.tile([128, CHUNK], F32, tag="qps")
                        for ci in range(CI):
                            nc.tensor.matmul(
                                qps[:], wq_sb[:, ci, co * 128:(co + 1) * 128],
                                xq_sb[:, ci, :], start=(ci == 0), stop=(ci == CI - 1))
                        nc.vector.tensor_copy(
                            qt_sb[:, co, qch * CHUNK:(qch + 1) * CHUNK], qps[:])

            # wo on the gpsimd (SWDGE) queue: latency-insensitive, keeps the
            # HW-DGE queues free for phase-B kt/v streams
            for ci in range(CI):
                nc.gpsimd.dma_start(wo_sb[:, ci, :], w3["wot"][:, ci, :])

            # ================= Phase B: attention + out-proj ================
            with tc.tile_pool(name="et", bufs=1) as etp, \
                 tc.tile_pool(name="ktb", bufs=4) as ktb_p, \
                 tc.tile_pool(name="vco", bufs=3) as vsp, \
                 tc.tile_pool(name="ysb", bufs=2) as ysb_p, \
                 tc.tile_pool(name="wb", bufs=2) as wb, \
                 tc.tile_pool(name="zst", bufs=3) as zstp, \
                 tc.tile_pool(name="ps_s", bufs=3, space="PSUM") as ps_s, \
                 tc.tile_pool(name="ps_sh", bufs=1, space="PSUM") as ps_sh, \
                 tc.tile_pool(name="ps_y", bufs=2, space="PSUM") as ps_y, \
                 tc.tile_pool(name="ps_z", bufs=2, space="PSUM") as ps_z:
                for ch in range(NCHUNK):
                    nkb = SLOT_KBS[ch]
                    rsl = slice(ch * CHUNK, (ch + 1) * CHUNK)
                    ri_b = wb.tile([128, CHUNK], F32, tag="rib")
                    nc.sync.dma_start(ri_b[:], rowidxb[:, rsl])

                    et = etp.tile([128, KB, CHUNK], F32R, tag="et")
                    # --- sweep 1a: scores + exp + mask (descending kb) ---
                    for p in range(nkb):
                        kb = KB - 1 - p
                        kt_b = ktb_p.tile([128, CI, 128], F32R, tag="ktb")
                        nc.scalar.dma_start(
                            kt_b[:], kt_d[:, :, kb * 128:(kb + 1) * 128])
                        sps = ps_s.tile([128, CHUNK], F32, tag="sps")
                        for ci in range(CI):
                            nc.tensor.matmul(
                                sps[:], kt_b[:, ci, :], qt_sb[:, ci, rsl],
                                start=(ci == 0), stop=(ci == CI - 1))
                        nc.scalar.activation(et[:, p, :], sps[:],
                                             mybir.ActivationFunctionType.Exp,
                                             scale=inv_sqrt_c)
                        mask = wb.tile([128, CHUNK], F32, tag="mask")
                        nc.vector.tensor_tensor(
                            mask[:], ki16_sb[:, kb:kb + 1].to_broadcast((128, CHUNK)),
                            ri_b[:], mybir.AluOpType.is_ge)
                        nc.vector.tensor_tensor(et[:, p, :], et[:, p, :], mask[:],
                                                mybir.AluOpType.mult)
                    # --- sweep 1b: key-sums via ones matmul ---
                    sums_ps = ps_sh.tile([1, CHUNK], F32, tag="shared")
                    for p in range(nkb):
                        nc.tensor.matmul(sums_ps[:], ones_128x1[:], et[:, p, :],
                                         start=(p == 0), stop=(p == nkb - 1))
                    recip = wb.tile([1, CHUNK], F32R, tag="recip")
                    with nc.allow_low_precision(reason="fp32r normalizer broadcast"):
                        nc.vector.reciprocal(recip[:], sums_ps[:])
                    rb_ps = ps_sh.tile([128, CHUNK], F32, tag="shared")
                    nc.tensor.matmul(rb_ps[:], ones_1x128[:], recip[:],
                                     start=True, stop=True)
                    rb_sb = wb.tile([128, CHUNK], F32, tag="rbsb")
                    nc.vector.tensor_copy(rb_sb[:], rb_ps[:])

                    # --- sweep 2: Y^T = V^T @ E^T per cout block ---
                    y_sb = ysb_p.tile([128, CO, CHUNK], F32R, tag="ysb")
                    for co in range(CO):
                        v_co = vsp.tile([128, KB, 128], F32R, tag="vco")
                        nc.sync.dma_start(
                            v_co[:, :nkb, :],
                            v_d[:, KB - nkb:, co * 128:(co + 1) * 128])
                        yps = ps_y.tile([128, CHUNK], F32, tag="yps")
                        for p in range(nkb):
                            nc.tensor.matmul(yps[:], v_co[:, nkb - 1 - p, :],
                                             et[:, p, :],
                                             start=(p == 0), stop=(p == nkb - 1))
                        nc.vector.tensor_copy(y_sb[:, co, :], yps[:])

                    # --- out-proj + normalize ---
                    for co in range(CO):
                        zps = ps_z.tile([128, CHUNK], F32, tag="zps")
                        for ci in range(CI):
                            nc.tensor.matmul(
                                zps[:], wo_sb[:, ci, co * 128:(co + 1) * 128],
                                y_sb[:, ci, :], start=(ci == 0), stop=(ci == CI - 1))
                        zst = zstp.tile([128, CHUNK], F32, tag="zst")
                        nc.vector.tensor_tensor(zst[:], zps[:], rb_sb[:],
                                                mybir.AluOpType.mult)
                        nc.sync.dma_start(zt[co * 128:(co + 1) * 128, rsl], zst[:])
    nc.compile()
    _NC_CACHE["nc"] = nc
    return nc


def make_in_maps(inputs):
    x = np.asarray(inputs["x"], dtype=np.float32)
    for bname in ("bq", "bk", "bv", "bo"):
        bval = np.asarray(inputs[bname])
        assert np.all(bval == 0.0), f"{bname} nonzero: unsupported fast path"
    wqt = np.ascontiguousarray(np.asarray(inputs["Wq"], np.float32).T)
    wkt = np.ascontiguousarray(np.asarray(inputs["Wk"], np.float32).T)
    wvt = np.ascontiguousarray(np.asarray(inputs["Wv"], np.float32).T)
    wot = np.ascontiguousarray(np.asarray(inputs["Wo"], np.float32).T)
    keyidx16 = (np.arange(T, dtype=np.float32).reshape(KB, 128).T + WINDOW
                ).copy()  # [128, KB]
    chunk_map = {0: (0, 3), 1: (1, 2)}  # slot 0 = denser chunk
    in_maps = []
    for core in range(N_CORES):
        b, h = divmod(core, 2)
        xt_b = np.ascontiguousarray(x[b].T)  # [C, T]
        ch0, ch1 = chunk_map[h]
        xtq = np.concatenate(
            [xt_b[:, ch0 * CHUNK:(ch0 + 1) * CHUNK],
             xt_b[:, ch1 * CHUNK:(ch1 + 1) * CHUNK]], axis=1)
        rowidx = np.concatenate(
            [np.arange(ch0 * CHUNK, (ch0 + 1) * CHUNK, dtype=np.float32),
             np.arange(ch1 * CHUNK, (ch1 + 1) * CHUNK, dtype=np.float32)])
        rowidxb = np.ascontiguousarray(
            np.broadcast_to(rowidx[None, :], (128, TOWN)))
        in_maps.append({
            "xt": xt_b, "xtq": np.ascontiguousarray(xtq),
            "wqt": wqt, "wkt": wkt, "wvt": wvt, "wot": wot,
            "keyidx16": keyidx16, "rowidxb": rowidxb,
        })
    return in_maps


def gather_output(results, dtype):
    out = np.empty((B, T, C), dtype=dtype)
    chunk_map = {0: (0, 3), 1: (1, 2)}
    for core in range(N_CORES):
        b, h = divmod(core, 2)
        y = results[core]["zt"].T  # [TOWN rows, C]
        ch0, ch1 = chunk_map[h]
        out[b, ch0 * CHUNK:(ch0 + 1) * CHUNK] = y[:CHUNK]
        out[b, ch1 * CHUNK:(ch1 + 1) * CHUNK] = y[CHUNK:]
    return out


def kernel(**inputs):
    nc = build()
    in_maps = make_in_maps(inputs)
    res = bass_utils.run_bass_kernel_spmd(nc, in_maps,
                                          core_ids=list(range(N_CORES)))
    return gather_output(res.results, np.asarray(inputs["x"]).dtype)



# revision 14
# speedup vs baseline: 1.3477x; 1.3477x over previous
"""TRN2 Bass kernel for nn_LocalAttention (B=4, T=2048, C=1024, window=16).

Sharding: 8 cores = (batch b, half h). Each core computes K^T/V for its
whole batch (duplicated across the 2 cores of a batch) and attention +
projections for its own 1024 rows, held as 4 slots of 256 rows.

All matmuls run in bf16 (error ~5e-3 vs the 2e-2 gate; same PE stream
rate as fp32r but ~4x faster LDWEIGHTS via FWL, half the DMA/SBUF).
PSUM accumulation is fp32. K^T, V, Q^T, E are SBUF-resident - no DRAM
scratch round-trip.

Orientation trick: host passes X^T and W^T so every matmul is natural:
  K^T = (Wk^T)^T @ X^T        [C, T]     (SBUF resident)
  V   = (X^T)^T @ Wv^T        [T, C]     (SBUF resident)
  Q^T = (Wq^T)^T @ Xq^T       [C, 1024]  (SBUF resident)
  S^T = (K^T_blk)^T @ Q^T     -> [keys, rows]; softmax-over-keys via
        ones-vector matmul partition reduction; E^T feeds
  Y^T = V_blk^T @ E^T            [C, rows]
  Z^T = (Wo^T)^T @ Y^T           [C, rows]

Sparsity: mask keeps j >= i - 16 (reverse-causal): row group g of 256
rows (g=0..7 per batch) needs only the last [16,15,13,11,9,7,5,3][g]
key blocks. Core h owns groups {2j + h}; slot j key counts
SLOT_NKB = (16,13,9,5) cover both cores' groups uniformly (43 block
positions/core vs 50 in the 512-row scheme). Descending kb order makes
each kept set a static prefix; the data-driven is_ge mask zeroes the
over-included tail. Mask applied multiplicatively post-exp (scores are
O(6), no overflow without max-subtraction).

Interleaved accumulation chains (V halves, Q halves, Z halves, Y slots,
S pieces) put consecutive matmuls on the same stationary operand so the
PE's weight double-buffer amortizes LDWEIGHTS.
"""
import numpy as np
import ml_dtypes

import concourse.bass as bass
import concourse.mybir as mybir
import concourse.tile as tile
from concourse import bacc
from concourse import bass_utils

N_CORES = 8
B, T, C = 4, 2048, 1024
WINDOW = 16
TOWN = T // 2          # own rows per core
SR = 256               # rows per slot
NSLOT = TOWN // SR     # 4
CI = C // 128          # 8 contraction blocks
CO = C // 128          # 8 output blocks
KB = T // 128          # 16 key blocks
TCH = 4                # t-chunks in phase A
CHUNK = T // TCH       # 512
SLOT_NKB = (16, 13, 9, 5)  # key-block positions per slot (descending kb)
F32 = mybir.dt.float32
F32R = mybir.dt.float32r
BF16 = mybir.dt.bfloat16
BF = ml_dtypes.bfloat16

_NC_CACHE = {}


def _width(p):
    """Active row width at key position p (slots packed left, sorted desc)."""
    return SR * sum(1 for n in SLOT_NKB if n > p)


def build():
    if "nc" in _NC_CACHE:
        return _NC_CACHE["nc"]
    nc = bacc.Bacc("TRN2", target_bir_lowering=False, debug=False,
                   num_devices=N_CORES)
    # Host pre-rearranges everything so each DMA reads contiguous >=2KB
    # per-partition lines (see make_in_maps).
    xt = nc.dram_tensor("xt", [128, TCH, CI, CHUNK], BF16,
                        kind="ExternalInput").ap()
    xtq = nc.dram_tensor("xtq", [128, CI, TOWN], BF16,
                         kind="ExternalInput").ap()
    wq_d = nc.dram_tensor("wq4", [128, CO, CI, 128], BF16,
                          kind="ExternalInput").ap()
    wk_d = nc.dram_tensor("wk4", [128, CO, CI, 128], BF16,
                          kind="ExternalInput").ap()
    wv_d = nc.dram_tensor("wv3", [128, CI, C], BF16,
                          kind="ExternalInput").ap()
    wo_d = nc.dram_tensor("wo4", [128, CO, CI, 128], BF16,
                          kind="ExternalInput").ap()
    keyidx16 = nc.dram_tensor("keyidx16", [128, KB], F32, kind="ExternalInput").ap()
    rowidxb = nc.dram_tensor("rowidxb", [128, TOWN], F32, kind="ExternalInput").ap()
    zt = nc.dram_tensor("zt", [C, TOWN], F32, kind="ExternalOutput").ap()

    inv_sqrt_c = float(1.0 / np.sqrt(C))

    with tile.TileContext(nc) as tc:
        with tc.tile_pool(name="res", bufs=1) as res:
            kt_sb = res.tile([128, CI, T], BF16, tag="kt")     # K^T resident
            v_sb = res.tile([128, KB, C], BF16, tag="v")       # V resident
            qt_sb = res.tile([128, CI, TOWN], BF16, tag="qt")  # Q^T resident
            ki16_sb = res.tile([128, KB], F32, tag="ki16")
            ri_b = res.tile([128, TOWN], F32, tag="rib")
            rb_sb = res.tile([128, TOWN], F32, tag="rbsb")
            ones_row_f32 = res.tile([1, 128], F32, tag="onesrf")
            nc.vector.memset(ones_row_f32[:], 1.0)
            ones_1x128 = res.tile([1, 128], F32R, tag="o1")
            nc.vector.tensor_copy(ones_1x128[:], ones_row_f32[:])
            ones_col_f32 = res.tile([128, 1], F32, tag="onescf")
            nc.vector.memset(ones_col_f32[:], 1.0)
            ones_128x1 = res.tile([128, 1], BF16, tag="o2")
            nc.vector.tensor_copy(ones_128x1[:], ones_col_f32[:])

            # ============ Phase A: projections ============
            with tc.tile_pool(name="wts", bufs=1) as wts, \
                 tc.tile_pool(name="xa", bufs=TCH) as xa, \
                 tc.tile_pool(name="ps_k", bufs=2, space="PSUM") as ps_k, \
                 tc.tile_pool(name="ps_v", bufs=2, space="PSUM") as ps_v, \
                 tc.tile_pool(name="ps_q", bufs=1, space="PSUM") as ps_q:
                wk_sb = wts.tile([128, CO, CI, 128], BF16, tag="wk")
                wv_sb = wts.tile([128, CI, C], BF16, tag="wv")
                wq_sb = wts.tile([128, CO, CI, 128], BF16, tag="wq")
                xtq_sb = wts.tile([128, CI, TOWN], BF16, tag="xtq")
                # first wk column + first xt chunk lead the sync queue
                nc.sync.dma_start(wk_sb[:, 0], wk_d[:, 0])
                xt_sbs = {}
                for tch in reversed(range(TCH)):
                    x_t = xa.tile([128, CI, CHUNK], BF16, tag="xa")
                    nc.sync.dma_start(x_t[:], xt[:, tch])
                    xt_sbs[tch] = x_t
                for co in range(1, CO):
                    nc.sync.dma_start(wk_sb[:, co], wk_d[:, co])
                for ci in range(CI):
                    nc.scalar.dma_start(wv_sb[:, ci, :], wv_d[:, ci, :])
                for co in range(CO):
                    nc.scalar.dma_start(wq_sb[:, co], wq_d[:, co])
                nc.scalar.dma_start(xtq_sb[:], xtq[:])
                nc.gpsimd.dma_start(ki16_sb[:], keyidx16[:])
                nc.gpsimd.dma_start(ri_b[:], rowidxb[:])

                for tch in reversed(range(TCH)):
                    xt_sb = xt_sbs[tch]
                    # K^T [cout, t]
                    for co in range(CO):
                        kps = ps_k.tile([128, CHUNK], F32, tag="kps")
                        for ci in range(CI):
                            nc.tensor.matmul(
                                kps[:], wk_sb[:, co, ci, :], xt_sb[:, ci, :],
                                start=(ci == 0), stop=(ci == CI - 1))
                        nc.vector.tensor_copy(
                            kt_sb[:, co, tch * CHUNK:(tch + 1) * CHUNK], kps[:])
                    # V [t, cout] - halves interleaved to share xt stationary
                    for tb in range(CHUNK // 128):
                        vps0 = ps_v.tile([128, 512], F32, tag="vps0")
                        vps1 = ps_v.tile([128, 512], F32, tag="vps1")
                        for ci in range(CI):
                            nc.tensor.matmul(
                                vps0[:], xt_sb[:, ci, tb * 128:(tb + 1) * 128],
                                wv_sb[:, ci, 0:512],
                                start=(ci == 0), stop=(ci == CI - 1))
                            nc.tensor.matmul(
                                vps1[:], xt_sb[:, ci, tb * 128:(tb + 1) * 128],
                                wv_sb[:, ci, 512:1024],
                                start=(ci == 0), stop=(ci == CI - 1))
                        tbg = tch * (CHUNK // 128) + tb
                        nc.vector.tensor_copy(v_sb[:, tbg, 0:512], vps0[:])
                        nc.vector.tensor_copy(v_sb[:, tbg, 512:1024], vps1[:])

                # Q^T - halves interleaved to share wq stationary
                for co in range(CO):
                    qps0 = ps_q.tile([128, 512], F32, tag="qps0")
                    qps1 = ps_q.tile([128, 512], F32, tag="qps1")
                    for ci in range(CI):
                        nc.tensor.matmul(
                            qps0[:], wq_sb[:, co, ci, :], xtq_sb[:, ci, 0:512],
                            start=(ci == 0), stop=(ci == CI - 1))
                        nc.tensor.matmul(
                            qps1[:], wq_sb[:, co, ci, :], xtq_sb[:, ci, 512:1024],
                            start=(ci == 0), stop=(ci == CI - 1))
                    nc.vector.tensor_copy(qt_sb[:, co, 0:512], qps0[:])
                    nc.vector.tensor_copy(qt_sb[:, co, 512:1024], qps1[:])

            # ============ Phase B: attention + out-proj ============
            with tc.tile_pool(name="bres", bufs=1) as bres, \
                 tc.tile_pool(name="wb", bufs=4) as wb:
                wo_sb = bres.tile([128, CO, CI, 128], BF16, tag="wo")
                # wo on gpsimd (SWDGE): needed only ~100us from now at Z
                for co in range(CO):
                    nc.gpsimd.dma_start(wo_sb[:, co], wo_d[:, co])
                et = bres.tile([128, KB, TOWN], BF16, tag="et")
                recip = bres.tile([1, TOWN], F32R, tag="recip")

                # --- sweep 1: scores + exp + mask + key-sums (desc. kb) ---
                with tc.tile_pool(name="ps_sum", bufs=1, space="PSUM") as ps_sum:
                    sums_a = ps_sum.tile([1, 512], F32, tag="suma")  # slots 0-1
                    sums_b = ps_sum.tile([1, 512], F32, tag="sumb")  # slots 2-3

                    def emit_sums(p):
                        # ones-matmul partition reduction, 2 positions behind
                        # the S sweep so exp/mask are long done
                        w = _width(p)
                        nc.tensor.matmul(
                            sums_a[:, :min(512, w)], ones_128x1[:],
                            et[:, p, 0:min(512, w)],
                            start=(p == 0), stop=(p == KB - 1),
                            skip_group_check=True)
                        if w > 512:
                            nc.tensor.matmul(
                                sums_b[:, :w - 512], ones_128x1[:],
                                et[:, p, 512:w],
                                start=(p == 0), stop=(p == SLOT_NKB[2] - 1),
                                skip_group_check=True)

                    with tc.tile_pool(name="ps_s", bufs=3, space="PSUM") as ps_s:
                        for p in range(KB):
                            kb = KB - 1 - p
                            w = _width(p)
                            pieces = [(0, min(512, w))]
                            if w > 512:
                                pieces.append((512, w - 512))
                            # interleave piece chains to share kt stationary
                            spss = [ps_s.tile([128, 512], F32, tag="sps",
                                              name=f"sps_{p}_{i}")
                                    for i in range(len(pieces))]
                            for ci in range(CI):
                                for (off, pw), sps in zip(pieces, spss):
                                    nc.tensor.matmul(
                                        sps[:, :pw],
                                        kt_sb[:, ci, kb * 128:(kb + 1) * 128],
                                        qt_sb[:, ci, off:off + pw],
                                        start=(ci == 0), stop=(ci == CI - 1))
                            for (off, pw), sps in zip(pieces, spss):
                                nc.scalar.activation(
                                    et[:, p, off:off + pw], sps[:, :pw],
                                    mybir.ActivationFunctionType.Exp,
                                    scale=inv_sqrt_c)
                                mask = wb.tile([128, 512], F32, tag="mask")
                                nc.vector.tensor_tensor(
                                    mask[:, :pw],
                                    ki16_sb[:, kb:kb + 1].to_broadcast((128, pw)),
                                    ri_b[:, off:off + pw], mybir.AluOpType.is_ge)
                                nc.vector.tensor_tensor(
                                    et[:, p, off:off + pw], et[:, p, off:off + pw],
                                    mask[:, :pw], mybir.AluOpType.mult)
                            if p >= 2:
                                emit_sums(p - 2)
                        emit_sums(KB - 2)
                        emit_sums(KB - 1)

                    with nc.allow_low_precision(reason="fp32r normalizer"):
                        nc.vector.reciprocal(recip[:, 0:512], sums_a[:])
                        nc.vector.reciprocal(recip[:, 512:1024], sums_b[:])

                # --- normalizer broadcast to all partitions via ones MM ---
                with tc.tile_pool(name="ps_rb", bufs=2, space="PSUM") as ps_rb:
                    for half in range(2):
                        rb_ps = ps_rb.tile([128, 512], F32, tag="rbps")
                        nc.tensor.matmul(rb_ps[:], ones_1x128[:],
                                         recip[:, half * 512:(half + 1) * 512],
                                         start=True, stop=True)
                        nc.vector.tensor_copy(
                            rb_sb[:, half * 512:(half + 1) * 512], rb_ps[:])

                # --- sweep 2: Y^T = V^T @ E^T; slot chains share v ---
                y_sb = bres.tile([128, CO, TOWN], BF16, tag="ysb")
                with tc.tile_pool(name="ps_y", bufs=2, space="PSUM") as ps_y:
                    for co in range(CO):
                        yps = [ps_y.tile([128, SR], F32, tag=f"yps{j}",
                                         name=f"yps_{co}_{j}")
                               for j in range(NSLOT)]
                        for p in range(KB):
                            kb = KB - 1 - p
                            for j in range(NSLOT):
                                if SLOT_NKB[j] > p:
                                    nc.tensor.matmul(
                                        yps[j][:],
                                        v_sb[:, kb, co * 128:(co + 1) * 128],
                                        et[:, p, j * SR:(j + 1) * SR],
                                        start=(p == 0),
                                        stop=(p == SLOT_NKB[j] - 1))
                        for j in range(NSLOT):
                            nc.vector.tensor_copy(
                                y_sb[:, co, j * SR:(j + 1) * SR], yps[j][:])

                # --- out-proj + normalize; halves share wo stationary ---
                with tc.tile_pool(name="zst", bufs=4) as zstp, \
                     tc.tile_pool(name="ps_z", bufs=2, space="PSUM") as ps_z:
                    for co in range(CO):
                        zps0 = ps_z.tile([128, 512], F32, tag="zps0")
                        zps1 = ps_z.tile([128, 512], F32, tag="zps1")
                        for ci in range(CI):
                            nc.tensor.matmul(
                                zps0[:], wo_sb[:, co, ci, :], y_sb[:, ci, 0:512],
                                start=(ci == 0), stop=(ci == CI - 1))
                            nc.tensor.matmul(
                                zps1[:], wo_sb[:, co, ci, :], y_sb[:, ci, 512:1024],
                                start=(ci == 0), stop=(ci == CI - 1))
                        for half, zps in ((0, zps0), (1, zps1)):
                            zst = zstp.tile([128, 512], F32, tag="zst")
                            nc.vector.tensor_tensor(
                                zst[:], zps[:],
                                rb_sb[:, half * 512:(half + 1) * 512],
                                mybir.AluOpType.mult)
                            nc.sync.dma_start(
                                zt[co * 128:(co + 1) * 128,
                                   half * 512:(half + 1) * 512], zst[:])
    nc.compile()
    _NC_CACHE["nc"] = nc
    return nc


def _w4(w):
    """W [C_out, C_in] -> lhsT-layout [128, CO, CI, 128] bf16 (2KB lines)."""
    wt = np.asarray(w, np.float32).T.astype(BF)  # [C_in, C_out]
    return np.ascontiguousarray(
        wt.reshape(CI, 128, CO, 128).transpose(1, 2, 0, 3))


def make_in_maps(inputs):
    x = np.asarray(inputs["x"], dtype=np.float32)
    for bname in ("bq", "bk", "bv", "bo"):
        bval = np.asarray(inputs[bname])
        assert np.all(bval == 0.0), f"{bname} nonzero: unsupported fast path"
    wq4 = _w4(inputs["Wq"])
    wk4 = _w4(inputs["Wk"])
    wo4 = _w4(inputs["Wo"])
    wvt = np.asarray(inputs["Wv"], np.float32).T.astype(BF)  # [C_in, C_out]
    wv3 = np.ascontiguousarray(wvt.reshape(CI, 128, C).transpose(1, 0, 2))
    keyidx16 = (np.arange(T, dtype=np.float32).reshape(KB, 128).T + WINDOW
                ).copy()  # [128, KB]
    in_maps = []
    for core in range(N_CORES):
        b, h = divmod(core, 2)
        xt_b = x[b].T.astype(BF)  # [C, T]
        xt4 = np.ascontiguousarray(
            xt_b.reshape(CI, 128, TCH, CHUNK).transpose(1, 2, 0, 3))
        groups = [2 * j + h for j in range(NSLOT)]
        xtq = np.concatenate(
            [xt_b[:, g * SR:(g + 1) * SR] for g in groups], axis=1)
        xtq3 = np.ascontiguousarray(
            xtq.reshape(CI, 128, TOWN).transpose(1, 0, 2))
        rowidx = np.concatenate(
            [np.arange(g * SR, (g + 1) * SR, dtype=np.float32) for g in groups])
        rowidxb = np.ascontiguousarray(
            np.broadcast_to(rowidx[None, :], (128, TOWN)))
        in_maps.append({
            "xt": xt4, "xtq": xtq3,
            "wq4": wq4, "wk4": wk4, "wv3": wv3, "wo4": wo4,
            "keyidx16": keyidx16, "rowidxb": rowidxb,
        })
    return in_maps


def gather_output(results, dtype):
    out = np.empty((B, T, C), dtype=dtype)
    for core in range(N_CORES):
        b, h = divmod(core, 2)
        y = results[core]["zt"].T  # [TOWN rows, C]
        for j in range(NSLOT):
            g = 2 * j + h
            out[b, g * SR:(g + 1) * SR] = y[j * SR:(j + 1) * SR]
    return out


def kernel(**inputs):
    nc = build()
    in_maps = make_in_maps(inputs)
    res = bass_utils.run_bass_kernel_spmd(nc, in_maps,
                                          core_ids=list(range(N_CORES)))
    return gather_output(res.results, np.asarray(inputs["x"]).dtype)


# revision 16
# speedup vs baseline: 1.4568x; 1.0809x over previous
"""TRN2 Bass kernel for nn_LocalAttention (B=4, T=2048, C=1024, window=16).

Sharding: 8 cores = (batch b, half h). Each core computes K^T/V for its
whole batch (duplicated across the 2 cores of a batch) and attention +
projections for its own 1024 rows, held as 4 slots of 256 rows.

All matmuls run in bf16 (error ~5e-3 vs the 2e-2 gate; same PE stream
rate as fp32r but ~4x faster LDWEIGHTS via FWL, half the DMA/SBUF).
PSUM accumulation is fp32. K^T, V, Q^T, E are SBUF-resident - no DRAM
scratch round-trip.

Orientation trick: host passes X^T and W^T so every matmul is natural:
  K^T = (Wk^T)^T @ X^T        [C, T]     (SBUF resident)
  V   = (X^T)^T @ Wv^T        [T, C]     (SBUF resident)
  Q^T = (Wq^T)^T @ Xq^T       [C, 1024]  (SBUF resident)
  S^T = (K^T_blk)^T @ Q^T     -> [keys, rows]; softmax-over-keys via
        ones-vector matmul partition reduction; E^T feeds
  Y^T = V_blk^T @ E^T            [C, rows]
  Z^T = (Wo^T)^T @ Y^T           [C, rows]

Sparsity: mask keeps j >= i - 16 (reverse-causal): row group g of 256
rows (g=0..7 per batch) needs only the last [16,15,13,11,9,7,5,3][g]
key blocks. Core h owns groups {2j + h}; slot j key counts
SLOT_NKB = (16,13,9,5) cover both cores' groups uniformly (43 block
positions/core vs 50 in the 512-row scheme). Descending kb order makes
each kept set a static prefix; the data-driven is_ge mask zeroes the
over-included tail. Mask applied multiplicatively post-exp (scores are
O(6), no overflow without max-subtraction).

Interleaved accumulation chains (V halves, Q halves, Z halves, Y slots,
S pieces) put consecutive matmuls on the same stationary operand so the
PE's weight double-buffer amortizes LDWEIGHTS.
"""
import numpy as np
import ml_dtypes

import concourse.bass as bass
import concourse.mybir as mybir
import concourse.tile as tile
from concourse import bacc
from concourse import bass_utils

N_CORES = 8
B, T, C = 4, 2048, 1024
WINDOW = 16
TOWN = T // 2          # own rows per core
SR = 256               # rows per slot
NSLOT = TOWN // SR     # 4
CI = C // 128          # 8 contraction blocks
CO = C // 128          # 8 output blocks
KB = T // 128          # 16 key blocks
TCH = 4                # t-chunks in phase A
CHUNK = T // TCH       # 512
SLOT_NKB = (16, 13, 9, 5)  # key-block positions per slot (descending kb)
F32 = mybir.dt.float32
F32R = mybir.dt.float32r
BF16 = mybir.dt.bfloat16
BF = ml_dtypes.bfloat16

_NC_CACHE = {}


def _width(p):
    """Active row width at key position p (slots packed left, sorted desc)."""
    return SR * sum(1 for n in SLOT_NKB if n > p)


def build():
    if "nc" in _NC_CACHE:
        return _NC_CACHE["nc"]
    nc = bacc.Bacc("TRN2", target_bir_lowering=False, debug=False,
                   num_devices=N_CORES)
    # Host pre-rearranges everything so each DMA reads contiguous >=2KB
    # per-partition lines (see make_in_maps).
    xt = nc.dram_tensor("xt", [128, TCH, CI, CHUNK], BF16,
                        kind="ExternalInput").ap()
    xtq = nc.dram_tensor("xtq", [128, CI, TOWN], BF16,
                         kind="ExternalInput").ap()
    wq_d = nc.dram_tensor("wq4", [128, CO, CI, 128], BF16,
                          kind="ExternalInput").ap()
    wk_d = nc.dram_tensor("wk4", [128, CO, CI, 128], BF16,
                          kind="ExternalInput").ap()
    wv_d = nc.dram_tensor("wv3", [128, CI, C], BF16,
                          kind="ExternalInput").ap()
    wo_d = nc.dram_tensor("wo4", [128, CO, CI, 128], BF16,
                          kind="ExternalInput").ap()
    keyidx16 = nc.dram_tensor("keyidx16", [128, KB], F32, kind="ExternalInput").ap()
    rowidxb = nc.dram_tensor("rowidxb", [128, TOWN], F32, kind="ExternalInput").ap()
    zt = nc.dram_tensor("zt", [C, TOWN], F32, kind="ExternalOutput").ap()

    inv_sqrt_c = float(1.0 / np.sqrt(C))

    with tile.TileContext(nc) as tc:
        with tc.tile_pool(name="res", bufs=1) as res:
            kt_sb = res.tile([128, CI, T], BF16, tag="kt")     # K^T resident
            v_sb = res.tile([128, KB, C], BF16, tag="v")       # V resident
            qt_sb = res.tile([128, CI, TOWN], BF16, tag="qt")  # Q^T resident
            ki16_sb = res.tile([128, KB], F32, tag="ki16")
            ri_b = res.tile([128, TOWN], F32, tag="rib")
            rb_sb = res.tile([128, TOWN], F32, tag="rbsb")
            ones_row_f32 = res.tile([1, 128], F32, tag="onesrf")
            nc.vector.memset(ones_row_f32[:], 1.0)
            ones_1x128 = res.tile([1, 128], F32R, tag="o1")
            nc.vector.tensor_copy(ones_1x128[:], ones_row_f32[:])
            ones_col_f32 = res.tile([128, 1], F32, tag="onescf")
            nc.vector.memset(ones_col_f32[:], 1.0)
            ones_128x1 = res.tile([128, 1], BF16, tag="o2")
            nc.vector.tensor_copy(ones_128x1[:], ones_col_f32[:])

            # ============ Phase A: projections ============
            with tc.tile_pool(name="wts", bufs=1) as wts, \
                 tc.tile_pool(name="xa", bufs=TCH) as xa, \
                 tc.tile_pool(name="ps_k", bufs=2, space="PSUM") as ps_k, \
                 tc.tile_pool(name="ps_v", bufs=2, space="PSUM") as ps_v, \
                 tc.tile_pool(name="ps_q", bufs=1, space="PSUM") as ps_q:
                wk_sb = wts.tile([128, CO, CI, 128], BF16, tag="wk")
                wv_sb = wts.tile([128, CI, C], BF16, tag="wv")
                wq_sb = wts.tile([128, CO, CI, 128], BF16, tag="wq")
                xtq_sb = wts.tile([128, CI, TOWN], BF16, tag="xtq")
                # Parallel queues: weights on sync, activations on scalar.
                # First K chain needs wk col0 + xt3 -> both lead their queue.
                for co in range(CO):
                    nc.sync.dma_start(wk_sb[:, co], wk_d[:, co])
                xt_sbs = {}
                for tch in (3,):
                    x_t3 = xa.tile([128, CI, CHUNK], BF16, tag="xa",
                                   name="xt_3")
                    nc.scalar.dma_start(x_t3[:], xt[:, tch])
                    xt_sbs[tch] = x_t3
                for ci in range(CI):
                    nc.scalar.dma_start(wv_sb[:, ci, :], wv_d[:, ci, :])
                for tch in (2, 1, 0):
                    x_t = xa.tile([128, CI, CHUNK], BF16, tag="xa",
                                  name=f"xt_{tch}")
                    nc.scalar.dma_start(x_t[:], xt[:, tch])
                    xt_sbs[tch] = x_t
                for co in range(CO):
                    nc.sync.dma_start(wq_sb[:, co], wq_d[:, co])
                nc.scalar.dma_start(xtq_sb[:], xtq[:])
                nc.gpsimd.dma_start(ki16_sb[:], keyidx16[:])
                nc.gpsimd.dma_start(ri_b[:], rowidxb[:])

                for tch in reversed(range(TCH)):
                    xt_sb = xt_sbs[tch]
                    # K^T [cout, t]
                    for co in range(CO):
                        kps = ps_k.tile([128, CHUNK], F32, tag="kps")
                        for ci in range(CI):
                            nc.tensor.matmul(
                                kps[:], wk_sb[:, co, ci, :], xt_sb[:, ci, :],
                                start=(ci == 0), stop=(ci == CI - 1))
                        nc.vector.tensor_copy(
                            kt_sb[:, co, tch * CHUNK:(tch + 1) * CHUNK], kps[:])
                    # V [t, cout] - halves interleaved to share xt stationary
                    for tb in range(CHUNK // 128):
                        vps0 = ps_v.tile([128, 512], F32, tag="vps0")
                        vps1 = ps_v.tile([128, 512], F32, tag="vps1")
                        for ci in range(CI):
                            nc.tensor.matmul(
                                vps0[:], xt_sb[:, ci, tb * 128:(tb + 1) * 128],
                                wv_sb[:, ci, 0:512],
                                start=(ci == 0), stop=(ci == CI - 1))
                            nc.tensor.matmul(
                                vps1[:], xt_sb[:, ci, tb * 128:(tb + 1) * 128],
                                wv_sb[:, ci, 512:1024],
                                start=(ci == 0), stop=(ci == CI - 1))
                        tbg = tch * (CHUNK // 128) + tb
                        nc.vector.tensor_copy(v_sb[:, tbg, 0:512], vps0[:])
                        nc.vector.tensor_copy(v_sb[:, tbg, 512:1024], vps1[:])

                # Q^T - halves interleaved to share wq stationary
                for co in range(CO):
                    qps0 = ps_q.tile([128, 512], F32, tag="qps0")
                    qps1 = ps_q.tile([128, 512], F32, tag="qps1")
                    for ci in range(CI):
                        nc.tensor.matmul(
                            qps0[:], wq_sb[:, co, ci, :], xtq_sb[:, ci, 0:512],
                            start=(ci == 0), stop=(ci == CI - 1))
                        nc.tensor.matmul(
                            qps1[:], wq_sb[:, co, ci, :], xtq_sb[:, ci, 512:1024],
                            start=(ci == 0), stop=(ci == CI - 1))
                    nc.vector.tensor_copy(qt_sb[:, co, 0:512], qps0[:])
                    nc.vector.tensor_copy(qt_sb[:, co, 512:1024], qps1[:])

            # ============ Phase B: attention + out-proj ============
            with tc.tile_pool(name="bres", bufs=1) as bres, \
                 tc.tile_pool(name="wb", bufs=4) as wb:
                wo_sb = bres.tile([128, CO, CI, 128], BF16, tag="wo")
                # wo on gpsimd (SWDGE): needed only ~100us from now at Z
                for co in range(CO):
                    nc.gpsimd.dma_start(wo_sb[:, co], wo_d[:, co])
                et = bres.tile([128, KB, TOWN], BF16, tag="et")
                recip = bres.tile([1, TOWN], F32R, tag="recip")

                # --- sweep 1: scores + exp + mask + key-sums (desc. kb) ---
                with tc.tile_pool(name="ps_sum", bufs=1, space="PSUM") as ps_sum:
                    sums_a = ps_sum.tile([1, 512], F32, tag="suma")  # slots 0-1
                    sums_b = ps_sum.tile([1, 512], F32, tag="sumb")  # slots 2-3

                    def emit_sums(p):
                        # ones-matmul partition reduction, 2 positions behind
                        # the S sweep so exp/mask are long done
                        w = _width(p)
                        nc.tensor.matmul(
                            sums_a[:, :min(512, w)], ones_128x1[:],
                            et[:, p, 0:min(512, w)],
                            start=(p == 0), stop=(p == KB - 1),
                            skip_group_check=True)
                        if w > 512:
                            nc.tensor.matmul(
                                sums_b[:, :w - 512], ones_128x1[:],
                                et[:, p, 512:w],
                                start=(p == 0), stop=(p == SLOT_NKB[2] - 1),
                                skip_group_check=True)

                    with tc.tile_pool(name="ps_s", bufs=3, space="PSUM") as ps_s:
                        for p in range(KB):
                            kb = KB - 1 - p
                            w = _width(p)
                            pieces = [(0, min(512, w))]
                            if w > 512:
                                pieces.append((512, w - 512))
                            # interleave piece chains to share kt stationary
                            spss = [ps_s.tile([128, 512], F32, tag="sps",
                                              name=f"sps_{p}_{i}")
                                    for i in range(len(pieces))]
                            for ci in range(CI):
                                for (off, pw), sps in zip(pieces, spss):
                                    nc.tensor.matmul(
                                        sps[:, :pw],
                                        kt_sb[:, ci, kb * 128:(kb + 1) * 128],
                                        qt_sb[:, ci, off:off + pw],
                                        start=(ci == 0), stop=(ci == CI - 1))
                            for (off, pw), sps in zip(pieces, spss):
                                nc.scalar.activation(
                                    et[:, p, off:off + pw], sps[:, :pw],
                                    mybir.ActivationFunctionType.Exp,
                                    scale=inv_sqrt_c)
                                mask = wb.tile([128, 512], F32, tag="mask")
                                nc.vector.tensor_tensor(
                                    mask[:, :pw],
                                    ki16_sb[:, kb:kb + 1].to_broadcast((128, pw)),
                                    ri_b[:, off:off + pw], mybir.AluOpType.is_ge)
                                nc.vector.tensor_tensor(
                                    et[:, p, off:off + pw], et[:, p, off:off + pw],
                                    mask[:, :pw], mybir.AluOpType.mult)
                            if p >= 2:
                                emit_sums(p - 2)
                        emit_sums(KB - 2)
                        emit_sums(KB - 1)

                    with nc.allow_low_precision(reason="fp32r normalizer"):
                        nc.vector.reciprocal(recip[:, 0:512], sums_a[:])
                        nc.vector.reciprocal(recip[:, 512:1024], sums_b[:])

                # --- normalizer broadcast to all partitions via ones MM ---
                with tc.tile_pool(name="ps_rb", bufs=2, space="PSUM") as ps_rb:
                    for half in range(2):
                        rb_ps = ps_rb.tile([128, 512], F32, tag="rbps")
                        nc.tensor.matmul(rb_ps[:], ones_1x128[:],
                                         recip[:, half * 512:(half + 1) * 512],
                                         start=True, stop=True)
                        nc.vector.tensor_copy(
                            rb_sb[:, half * 512:(half + 1) * 512], rb_ps[:])

                # --- sweep 2: Y^T = V^T @ E^T; slot chains share v ---
                y_sb = bres.tile([128, CO, TOWN], BF16, tag="ysb")
                with tc.tile_pool(name="ps_y", bufs=2, space="PSUM") as ps_y:
                    for co in range(CO):
                        yps = [ps_y.tile([128, SR], F32, tag=f"yps{j}",
                                         name=f"yps_{co}_{j}")
                               for j in range(NSLOT)]
                        for p in range(KB):
                            kb = KB - 1 - p
                            for j in range(NSLOT):
                                if SLOT_NKB[j] > p:
                                    nc.tensor.matmul(
                                        yps[j][:],
                                        v_sb[:, kb, co * 128:(co + 1) * 128],
                                        et[:, p, j * SR:(j + 1) * SR],
                                        start=(p == 0),
                                        stop=(p == SLOT_NKB[j] - 1))
                        for j in range(NSLOT):
                            nc.vector.tensor_copy(
                                y_sb[:, co, j * SR:(j + 1) * SR], yps[j][:])

                # --- out-proj + normalize; halves share wo stationary ---
                with tc.tile_pool(name="zst", bufs=4) as zstp, \
                     tc.tile_pool(name="ps_z", bufs=2, space="PSUM") as ps_z:
                    for co in range(CO):
                        zps0 = ps_z.tile([128, 512], F32, tag="zps0")
                        zps1 = ps_z.tile([128, 512], F32, tag="zps1")
                        for ci in range(CI):
                            nc.tensor.matmul(
                                zps0[:], wo_sb[:, co, ci, :], y_sb[:, ci, 0:512],
                                start=(ci == 0), stop=(ci == CI - 1))
                            nc.tensor.matmul(
                                zps1[:], wo_sb[:, co, ci, :], y_sb[:, ci, 512:1024],
                                start=(ci == 0), stop=(ci == CI - 1))
                        for half, zps in ((0, zps0), (1, zps1)):
                            zst = zstp.tile([128, 512], F32, tag="zst")
                            nc.vector.tensor_tensor(
                                zst[:], zps[:],
                                rb_sb[:, half * 512:(half + 1) * 512],
                                mybir.AluOpType.mult)
                            nc.sync.dma_start(
                                zt[co * 128:(co + 1) * 128,
                                   half * 512:(half + 1) * 512], zst[:])
    nc.compile()
    _NC_CACHE["nc"] = nc
    return nc


def _w4(w):
    """W [C_out, C_in] -> lhsT-layout [128, CO, CI, 128] bf16 (2KB lines)."""
    wt = np.asarray(w, np.float32).T.astype(BF)  # [C_in, C_out]
    return np.ascontiguousarray(
        wt.reshape(CI, 128, CO, 128).transpose(1, 2, 0, 3))


def make_in_maps(inputs):
    x = np.asarray(inputs["x"], dtype=np.float32)
    for bname in ("bq", "bk", "bv", "bo"):
        bval = np.asarray(inputs[bname])
        assert np.all(bval == 0.0), f"{bname} nonzero: unsupported fast path"
    wq4 = _w4(inputs["Wq"])
    wk4 = _w4(inputs["Wk"])
    wo4 = _w4(inputs["Wo"])
    wvt = np.asarray(inputs["Wv"], np.float32).T.astype(BF)  # [C_in, C_out]
    wv3 = np.ascontiguousarray(wvt.reshape(CI, 128, C).transpose(1, 0, 2))
    keyidx16 = (np.arange(T, dtype=np.float32).reshape(KB, 128).T + WINDOW
                ).copy()  # [128, KB]
    in_maps = []
    for core in range(N_CORES):
        b, h = divmod(core, 2)
        xt_b = x[b].T.astype(BF)  # [C, T]
        xt4 = np.ascontiguousarray(
            xt_b.reshape(CI, 128, TCH, CHUNK).transpose(1, 2, 0, 3))
        groups = [2 * j + h for j in range(NSLOT)]
        xtq = np.concatenate(
            [xt_b[:, g * SR:(g + 1) * SR] for g in groups], axis=1)
        xtq3 = np.ascontiguousarray(
            xtq.reshape(CI, 128, TOWN).transpose(1, 0, 2))
        rowidx = np.concatenate(
            [np.arange(g * SR, (g + 1) * SR, dtype=np.float32) for g in groups])
        rowidxb = np.ascontiguousarray(
            np.broadcast_to(rowidx[None, :], (128, TOWN)))
        in_maps.append({
            "xt": xt4, "xtq": xtq3,
            "wq4": wq4, "wk4": wk4, "wv3": wv3, "wo4": wo4,
            "keyidx16": keyidx16, "rowidxb": rowidxb,
        })
    return in_maps


def gather_output(results, dtype):
    out = np.empty((B, T, C), dtype=dtype)
    for core in range(N_CORES):
        b, h = divmod(core, 2)
        y = results[core]["zt"].T  # [TOWN rows, C]
        for j in range(NSLOT):
            g = 2 * j + h
            out[b, g * SR:(g + 1) * SR] = y[j * SR:(j + 1) * SR]
    return out


def kernel(**inputs):
    nc = build()
    in_maps = make_in_maps(inputs)
    res = bass_utils.run_bass_kernel_spmd(nc, in_maps,
                                          core_ids=list(range(N_CORES)))
    return gather_output(res.results, np.asarray(inputs["x"]).dtype)
